# revision 8
# baseline (speedup 1.0000x reference)
"""MDGAT sparse-attention block on 8 Trainium2 NeuronCores (Bass/Tile).

Sharding: data-parallel over batch — core b computes batch element b end-to-end
(no collectives). Inside a core: 4 heads x 16 n-tiles of the [2048, 2048]
score matrix.

Algorithm per (head, n-tile of 128 rows):
  1. PE: scores = q_tile^T k               [128n, 2048m] fp32 in PSUM
  2. ACT: evict scores to SBUF
  3. DVE: 4 rounds of (max8 -> match_replace(-1e30)) => topv [128,32] sorted
     descending (exactly jax.lax.top_k's values incl. tie order), and w4 =
     scores with the top-32 positions overwritten by -1e30.
  4. ACT: Z = sum exp(topv - rowmax) via accum_out; Ln for log-sum-exp.
  5. ACT: e = exp(scores - rowmax - lnZ)    (softmax fully folded into bias)
  6. DVE: es = (w4 == -1e30) * e            (one fused scalar_tensor_tensor)
  7. PE: transpose es in 128-col chunks; ACT evicts to SBUF
  8. PE: msgT[dh, n] = sum_m vT[m, dh]^T es^T[m, n]  (PSUM-accumulated)
Host-side weight preprocessing removes every on-chip shuffle: head interleave
permutation folded into Wq/Wk/Wv rows and Wm columns, 1/sqrt(dh) into Wq/bq,
v-bias into the merge bias, inference-BN into W1/b1.
"""

import numpy as np

B, D, H, N, M, K = 8, 128, 4, 2048, 2048, 32
DH = D // H
P = 128
NEG = -1.0e30

_CACHE = {}


def _build():
    import concourse.bacc as bacc
    import concourse.mybir as mybir
    import concourse.tile as tile
    from concourse.bass import ds, ts
    from concourse.masks import make_identity

    f32 = mybir.dt.float32
    AF = mybir.ActivationFunctionType
    OP = mybir.AluOpType

    nc = bacc.Bacc(
        "TRN2",
        target_bir_lowering=False,
        debug=False,
        enable_asserts=False,
        num_devices=8,
    )

    x_d = nc.dram_tensor("x", [P, N], f32, kind="ExternalInput").ap()
    src_d = nc.dram_tensor("src", [P, N], f32, kind="ExternalInput").ap()
    wqT_d = nc.dram_tensor("wqT", [P, P], f32, kind="ExternalInput").ap()
    wkT_d = nc.dram_tensor("wkT", [P, P], f32, kind="ExternalInput").ap()
    wvT_d = nc.dram_tensor("wvT", [P, P], f32, kind="ExternalInput").ap()
    wmT_d = nc.dram_tensor("wmT", [P, P], f32, kind="ExternalInput").ap()
    w1T_d = nc.dram_tensor("w1T", [P, 512], f32, kind="ExternalInput").ap()
    w2T_d = nc.dram_tensor("w2T", [P, 256], f32, kind="ExternalInput").ap()
    bias_d = nc.dram_tensor("biases", [P, 8], f32, kind="ExternalInput").ap()
    out_d = nc.dram_tensor("out", [P, N], f32, kind="ExternalOutput").ap()

    # bias column indices
    BQ, BK, BM, B1LO, B1HI, B2 = 0, 1, 2, 3, 4, 5

    with tile.TileContext(nc) as tc:
        with (
            tc.tile_pool(name="consts", bufs=1) as cp,
            tc.tile_pool(name="persist", bufs=1) as pp,
        ):
            ident = cp.tile([P, P], f32)
            make_identity(nc, ident)
            wqT = cp.tile([P, P], f32)
            nc.sync.dma_start(out=wqT, in_=wqT_d)
            wkT = cp.tile([P, P], f32)
            nc.sync.dma_start(out=wkT, in_=wkT_d)
            wvT = cp.tile([P, P], f32)
            nc.sync.dma_start(out=wvT, in_=wvT_d)
            wmT = cp.tile([P, P], f32)
            nc.sync.dma_start(out=wmT, in_=wmT_d)
            w1T = cp.tile([P, 512], f32)
            nc.sync.dma_start(out=w1T, in_=w1T_d)
            w2T = cp.tile([P, 256], f32)
            nc.sync.dma_start(out=w2T, in_=w2T_d)
            bia = cp.tile([P, 8], f32)
            nc.sync.dma_start(out=bia, in_=bias_d)

            x_sb = pp.tile([P, N], f32)
            nc.sync.dma_start(out=x_sb, in_=x_d)
            src_sb = pp.tile([P, N], f32)
            nc.sync.dma_start(out=src_sb, in_=src_d)
            q_sb = pp.tile([P, N], f32)
            k_sb = pp.tile([P, N], f32)
            # head 3 sits at base partition 96, which PE cannot address as a
            # matmul operand ({0,32,64} only) — DMA-shift it to partition 0.
            q3_sb = pp.tile([DH, N], f32)
            k3_sb = pp.tile([DH, N], f32)
            vt_sb = pp.tile([P, N], f32)  # col = mchunk*128 + (h*32+dh)
            mm_sb = pp.tile([P, N], f32)  # row = h*32+dh (permuted msg chans)

            # ---- Phase 1: projections ----
            with tc.tile_pool(name="p1ps", bufs=2, space="PSUM") as p1:
                for j in range(4):
                    ps = p1.tile([P, 512], f32, tag="pj")
                    nc.tensor.matmul(
                        ps, wqT, x_sb[:, ts(j, 512)], start=True, stop=True
                    )
                    nc.scalar.activation(
                        q_sb[:, ts(j, 512)], ps, AF.Identity, bias=bia[:, BQ : BQ + 1]
                    )
                for j in range(4):
                    ps = p1.tile([P, 512], f32, tag="pj")
                    nc.tensor.matmul(
                        ps, wkT, src_sb[:, ts(j, 512)], start=True, stop=True
                    )
                    nc.scalar.activation(
                        k_sb[:, ts(j, 512)], ps, AF.Identity, bias=bia[:, BK : BK + 1]
                    )
                nc.sync.dma_start(out=q3_sb, in_=q_sb[3 * DH : 4 * DH, :])
                nc.sync.dma_start(out=k3_sb, in_=k_sb[3 * DH : 4 * DH, :])
                # vT: out[m, o] = sum_c src[c, m] * WvT[c, o]  (no bias: folded)
                for g in range(4):
                    ps = p1.tile([P, 512], f32, tag="pj")
                    for c4 in range(4):
                        mc = g * 4 + c4
                        nc.tensor.matmul(
                            ps[:, ts(c4, P)],
                            src_sb[:, ts(mc, P)],
                            wvT,
                            start=True,
                            stop=True,
                        )
                    nc.scalar.activation(vt_sb[:, ts(g, 512)], ps, AF.Copy, bias=0.0)

            # ---- Phase 2: sparse attention per (h, n-tile) ----
            with (
                tc.tile_pool(name="scps", bufs=1, space="PSUM") as sp,
                tc.tile_pool(name="trps", bufs=2, space="PSUM") as tp,
                tc.tile_pool(name="mgps", bufs=2, space="PSUM") as mp,
                tc.tile_pool(name="attb", bufs=2) as ab,
                tc.tile_pool(name="attc", bufs=1) as ac,
                tc.tile_pool(name="smal", bufs=3) as sm,
            ):
                for h in range(H):
                    if h < 3:
                        hq = q_sb[h * DH : (h + 1) * DH, :]
                        hk = k_sb[h * DH : (h + 1) * DH, :]
                    else:
                        hq = q3_sb
                        hk = k3_sb
                    for nt in range(16):
                        ps_sc = sp.tile([P, M], f32, tag="sc")
                        for j in range(4):
                            nc.tensor.matmul(
                                ps_sc[:, ts(j, 512)],
                                hq[:, ts(nt, P)],
                                hk[:, ts(j, 512)],
                                start=True,
                                stop=True,
                            )
                        sc = ab.tile([P, M], f32, tag="sc_sb")
                        nc.scalar.activation(sc, ps_sc, AF.Copy, bias=0.0)

                        topv = sm.tile([P, 32], f32, tag="topv")
                        wa = ac.tile([P, M], f32, tag="wa")
                        wb = ac.tile([P, M], f32, tag="wb")
                        nc.vector.max(out=topv[:, 0:8], in_=sc)
                        nc.vector.match_replace(
                            out=wa, in_to_replace=topv[:, 0:8], in_values=sc,
                            imm_value=NEG,
                        )
                        nc.vector.max(out=topv[:, 8:16], in_=wa)
                        nc.vector.match_replace(
                            out=wb, in_to_replace=topv[:, 8:16], in_values=wa,
                            imm_value=NEG,
                        )
                        nc.vector.max(out=topv[:, 16:24], in_=wb)
                        nc.vector.match_replace(
                            out=wa, in_to_replace=topv[:, 16:24], in_values=wb,
                            imm_value=NEG,
                        )
                        nc.vector.max(out=topv[:, 24:32], in_=wa)
                        nc.vector.match_replace(
                            out=wb, in_to_replace=topv[:, 24:32], in_values=wa,
                            imm_value=NEG,
                        )

                        nrm = sm.tile([P, 1], f32, tag="nrm")
                        nc.vector.tensor_scalar_mul(nrm, topv[:, 0:1], -1.0)
                        etop = sm.tile([P, 32], f32, tag="etop")
                        zs = sm.tile([P, 1], f32, tag="zs")
                        nc.scalar.activation(
                            etop, topv, AF.Exp, bias=nrm, accum_out=zs
                        )
                        lnz = sm.tile([P, 1], f32, tag="lnz")
                        nc.scalar.activation(lnz, zs, AF.Ln)
                        b2v = sm.tile([P, 1], f32, tag="b2v")
                        nc.vector.tensor_sub(b2v, nrm, lnz)

                        e_sb = ac.tile([P, M], f32, tag="e")
                        nc.scalar.activation(e_sb, sc, AF.Exp, bias=b2v)
                        es = ab.tile([P, M], f32, tag="es")
                        nc.vector.scalar_tensor_tensor(
                            out=es, in0=wb, scalar=NEG, in1=e_sb,
                            op0=OP.is_equal, op1=OP.mult,
                        )

                        esT = ac.tile([P, M], f32, tag="esT")
                        for g in range(4):
                            pt = tp.tile([P, 512], f32, tag="tr")
                            for c4 in range(4):
                                nc.tensor.transpose(
                                    pt[:, ts(c4, P)], es[:, ts(g * 4 + c4, P)], ident
                                )
                            nc.scalar.activation(
                                esT[:, ts(g, 512)], pt, AF.Copy, bias=0.0
                            )

                        mg = mp.tile([DH, P], f32, tag="mg")
                        for c in range(16):
                            nc.tensor.matmul(
                                mg,
                                vt_sb[:, ds(c * P + h * DH, DH)],
                                esT[:, ts(c, P)],
                                start=(c == 0),
                                stop=(c == 15),
                            )
                        nc.scalar.activation(
                            mm_sb[h * DH : (h + 1) * DH, ts(nt, P)], mg,
                            AF.Copy, bias=0.0,
                        )

            # ---- Phase 3: merge + MLP ----
            with (
                tc.tile_pool(name="p3ps", bufs=2, space="PSUM") as p3,
                tc.tile_pool(name="p3sb", bufs=2) as s3,
            ):
                for j in range(4):
                    mps = p3.tile([P, 512], f32, tag="mrg")
                    nc.tensor.matmul(
                        mps, wmT, mm_sb[:, ts(j, 512)], start=True, stop=True
                    )
                    mrg = s3.tile([P, 512], f32, tag="mrgs")
                    nc.scalar.activation(
                        mrg, mps, AF.Identity, bias=bia[:, BM : BM + 1]
                    )
                    zlo = p3.tile([P, 512], f32, tag="zlo")
                    nc.tensor.matmul(
                        zlo, w1T[:, 0:128], x_sb[:, ts(j, 512)], start=True, stop=False
                    )
                    nc.tensor.matmul(
                        zlo, w1T[:, 256:384], mrg, start=False, stop=True
                    )
                    rlo = s3.tile([P, 512], f32, tag="rlo")
                    nc.scalar.activation(
                        rlo, zlo, AF.Relu, bias=bia[:, B1LO : B1LO + 1]
                    )
                    zhi = p3.tile([P, 512], f32, tag="zhi")
                    nc.tensor.matmul(
                        zhi, w1T[:, 128:256], x_sb[:, ts(j, 512)], start=True,
                        stop=False,
                    )
                    nc.tensor.matmul(
                        zhi, w1T[:, 384:512], mrg, start=False, stop=True
                    )
                    rhi = s3.tile([P, 512], f32, tag="rhi")
                    nc.scalar.activation(
                        rhi, zhi, AF.Relu, bias=bia[:, B1HI : B1HI + 1]
                    )
                    dps = p3.tile([P, 512], f32, tag="dl")
                    nc.tensor.matmul(dps, w2T[:, 0:128], rlo, start=True, stop=False)
                    nc.tensor.matmul(dps, w2T[:, 128:256], rhi, start=False, stop=True)
                    dsb = s3.tile([P, 512], f32, tag="dsb")
                    nc.scalar.activation(
                        dsb, dps, AF.Identity, bias=bia[:, B2 : B2 + 1]
                    )
                    nc.sync.dma_start(out=out_d[:, ts(j, 512)], in_=dsb)

    nc.compile()
    return nc


def _prep_host(inputs):
    perm = np.array([(r % DH) * H + (r // DH) for r in range(D)])
    s = np.float32(1.0 / np.sqrt(DH))
    Wq, bq = inputs["Wq"], inputs["bq"]
    Wk, bk = inputs["Wk"], inputs["bk"]
    Wv, bv = inputs["Wv"], inputs["bv"]
    Wm, bm = inputs["Wm"], inputs["bm"]
    W1, b1 = inputs["W1"], inputs["b1"]
    g1, beta1 = inputs["g1"], inputs["beta1"]
    mu1, var1 = inputs["mu1"], inputs["var1"]
    W2, b2 = inputs["W2"], inputs["b2"]

    f = np.float32
    c = np.ascontiguousarray
    wqT = c((Wq[perm] * s).T.astype(f))
    wkT = c(Wk[perm].T.astype(f))
    wvT = c(Wv[perm].T.astype(f))
    Wm_e = Wm[:, perm].astype(f)
    wmT = c(Wm_e.T)
    bm_e2 = (bm + Wm_e @ (bv[perm].astype(f))).astype(f)
    grs = (g1 / np.sqrt(var1 + 1e-5)).astype(f)
    W1_e = (W1 * grs[:, None]).astype(f)
    b1_e = ((b1 - mu1) * grs + beta1).astype(f)
    w1T = c(np.concatenate([W1_e[:, :128].T, W1_e[:, 128:].T], axis=1))
    w2T = c(np.concatenate([W2[:, :128].T, W2[:, 128:].T], axis=1).astype(f))
    biases = np.zeros((P, 8), f)
    biases[:, 0] = bq[perm] * s
    biases[:, 1] = bk[perm]
    biases[:, 2] = bm_e2
    biases[:, 3] = b1_e[:128]
    biases[:, 4] = b1_e[128:]
    biases[:, 5] = b2
    shared = {
        "wqT": wqT, "wkT": wkT, "wvT": wvT, "wmT": wmT,
        "w1T": w1T, "w2T": w2T, "biases": biases,
    }
    x = np.asarray(inputs["x"], f)
    src = np.asarray(inputs["source"], f)
    in_maps = [
        {"x": c(x[b]), "src": c(src[b]), **shared} for b in range(B)
    ]
    return in_maps


def _run(nc, in_maps, trace=False):
    from concourse import bass_utils

    return bass_utils.run_bass_kernel_spmd(
        nc, in_maps, core_ids=list(range(B)), trace=trace
    )


def kernel(**inputs) -> np.ndarray:
    if "nc" not in _CACHE:
        _CACHE["nc"] = _build()
    nc = _CACHE["nc"]
    in_maps = _prep_host(inputs)
    res = _run(nc, in_maps)
    out = np.stack([np.asarray(res.results[b]["out"]) for b in range(B)])
    return out.astype(np.float32)
